# revision 5
# baseline (speedup 1.0000x reference)
"""Sparse BERT self-attention (DeBERTa-style one-pass mask) on 8 Trainium2
NeuronCores. Data-parallel over batch: core b handles batch element b.

Shapes (hardcoded per problem spec):
  B=8, S=1408, D=768, H=12, Dh=64, L=64 (signal), CDD=20, T=128 (terms),
  AF = CDD*L = 1280.

Mask structure (training-mode one-pass, attention_mask==1 everywhere):
  - cdd query rows [0,1280): candidate c attends to its own 64 signal keys
    plus the 128 term keys  -> 192 keys per query.
  - term query rows [1280,1408): attend among the 128 term rows, with the
    *query* projection used for both sides (reference quirk).

Math notes (exact reassociations used by the kernel):
  - bk never enters: (Q+bq)·bk is constant over keys -> cancels in softmax.
  - bq IS added to Q (per-partition add in the Q^T layout).
  - bv is added after normalization (sum_k p = 1 -> +bv once).
  - exp without max-subtraction: |scores| <= ~5, safe in fp32 psum.
  - denominator: V tiles carry a ones-column per head; the ctx matmul
    accumulates sum(exp) into output column 64.
"""

import sys

sys.path.insert(0, "/opt/trn_rl_repo")

import numpy as np
import ml_dtypes

import concourse.bass as bass
import concourse.mybir as mybir
import concourse.tile as tile
from concourse.bass_utils import run_bass_kernel_spmd

# ---------------------------------------------------------------- constants
B, S, D = 8, 1408, 768
H, Dh = 12, 64
L, CDD, T = 64, 20, 128
AF = CDD * L  # 1280
NDC = D // 128  # 6 chunks of the contraction/output dim
NST = S // 128  # 11 s-tiles
SCALE = 1.0 / 8.0  # 1/sqrt(Dh)

BF16 = mybir.dt.bfloat16
F32 = mybir.dt.float32

QK_SCHUNKS = [(0, 512), (512, 1024), (1024, 1408)]  # s-chunks for Q/K proj
TERM_QCHUNKS = [(0, 512), (512, 1024), (1024, 1280)]  # cdd query chunks
V_OCHUNKS = [(0, 512), (512, 768)]  # output-dim chunks for V proj


# --------------------------------------------- walrus sem-wait legalization
def _legalize_waits(nc, max_waits=1):
    """This container's walrus rejects more than one sem wait per
    instruction. Hoist excess waits onto NOPs inserted just before the
    instruction on the same engine (engine streams execute in block order,
    so the conjunction of waits is preserved)."""
    from concourse import mybir

    k = 0
    for fn in nc.m.functions:
        for bb in fn.blocks:
            new_list = []
            changed = False
            for inst in bb.instructions:
                si = inst.sync_info
                waits = list(si.on_wait) if si is not None else []
                if len(waits) > max_waits:
                    changed = True
                    for w in waits[:-max_waits]:
                        nop = mybir.InstNoOp(name=f"waitsplit_{k}", ins=[], outs=[])
                        k += 1
                        nop.engine = inst.engine
                        nop.sync_info = mybir.SyncInfo(on_wait=[w], on_update=[])
                        new_list.append(nop)
                    inst.sync_info = mybir.SyncInfo(
                        on_wait=waits[-max_waits:], on_update=list(si.on_update)
                    )
                new_list.append(inst)
            if changed:
                bb.instructions = new_list


# ------------------------------------------------------------ bass program
def _build_program():
    nc = bass.Bass()
    AF_ = mybir.ActivationFunctionType
    ALU = mybir.AluOpType

    xT_d = nc.dram_tensor("xT", [D, S], BF16, kind="ExternalInput")
    wqT_d = nc.dram_tensor("wqT", [D, D], BF16, kind="ExternalInput")
    wkT_d = nc.dram_tensor("wkT", [D, D], BF16, kind="ExternalInput")
    wvT_d = nc.dram_tensor("wvT", [D, D], BF16, kind="ExternalInput")
    bq_d = nc.dram_tensor("bq", [NDC, 128, 1], F32, kind="ExternalInput")
    bvb_d = nc.dram_tensor("bvb", [128, D], F32, kind="ExternalInput")
    out_d = nc.dram_tensor("out", [S, D], F32, kind="ExternalOutput")

    with tile.TileContext(nc) as tc:
        with (
            tc.tile_pool(name="persist", bufs=1) as pp,
            tc.tile_pool(name="exps", bufs=2) as ep,
            tc.tile_pool(name="misc", bufs=4) as mp,
        ):
            # ---------------- input DMA
            xt = []
            for j in range(NDC):
                t = pp.tile([128, S], BF16, name=f"xt{j}", tag=f"xt{j}")
                nc.sync.dma_start(out=t, in_=xT_d[j * 128 : (j + 1) * 128, :])
                xt.append(t)
            wt = {}
            for nm, dram in (("q", wqT_d), ("k", wkT_d), ("v", wvT_d)):
                tiles = []
                for j in range(NDC):
                    t = pp.tile([128, D], BF16, name=f"w{nm}{j}", tag=f"w{nm}{j}")
                    nc.sync.dma_start(out=t, in_=dram[j * 128 : (j + 1) * 128, :])
                    tiles.append(t)
                wt[nm] = tiles
            bqt = []
            for j in range(NDC):
                t = pp.tile([128, 1], F32, name=f"bq{j}", tag=f"bq{j}")
                nc.sync.dma_start(out=t, in_=bq_d[j])
                bqt.append(t)
            bvb = pp.tile([128, D], F32, name="bvb", tag="bvb")
            nc.sync.dma_start(out=bvb, in_=bvb_d[:, :])

            QT = [pp.tile([128, S], BF16, name=f"qT{j}", tag=f"qT{j}") for j in range(NDC)]
            KT = [pp.tile([128, S], BF16, name=f"kT{j}", tag=f"kT{j}") for j in range(NDC)]
            # V tiles: [128, H, Dh+1]; column Dh holds ones (denominator).
            V = [pp.tile([128, H, Dh + 1], BF16, name=f"v{st}", tag=f"v{st}") for st in range(NST)]
            OUT = [pp.tile([128, D], F32, name=f"o{st}", tag=f"o{st}") for st in range(NST)]

            # ---------------- projections
            with tc.tile_pool(name="pproj", bufs=4, space=bass.MemorySpace.PSUM) as pj:
                for oc in range(NDC):
                    for s0, s1 in QK_SCHUNKS:
                        w = s1 - s0
                        pq = pj.tile([128, 512], F32, name="pq", tag="proj")
                        for dc in range(NDC):
                            nc.tensor.matmul(
                                pq[:, :w],
                                lhsT=wt["q"][dc][:, oc * 128 : (oc + 1) * 128],
                                rhs=xt[dc][:, s0:s1],
                                start=(dc == 0),
                                stop=(dc == NDC - 1),
                            )
                        # Q^T = psum + bq (per-partition), cast to bf16
                        nc.vector.tensor_scalar_add(
                            out=QT[oc][:, s0:s1], in0=pq[:, :w], scalar1=bqt[oc]
                        )
                        pk = pj.tile([128, 512], F32, name="pk", tag="proj")
                        for dc in range(NDC):
                            nc.tensor.matmul(
                                pk[:, :w],
                                lhsT=wt["k"][dc][:, oc * 128 : (oc + 1) * 128],
                                rhs=xt[dc][:, s0:s1],
                                start=(dc == 0),
                                stop=(dc == NDC - 1),
                            )
                        nc.scalar.activation(
                            out=KT[oc][:, s0:s1], in_=pk[:, :w], func=AF_.Copy
                        )
                for st in range(NST):
                    for o0, o1 in V_OCHUNKS:
                        w = o1 - o0
                        pv = pj.tile([128, 512], F32, name="pv", tag="proj")
                        for dc in range(NDC):
                            nc.tensor.matmul(
                                pv[:, :w],
                                lhsT=xt[dc][:, st * 128 : (st + 1) * 128],
                                rhs=wt["v"][dc][:, o0:o1],
                                start=(dc == 0),
                                stop=(dc == NDC - 1),
                            )
                        nh = w // Dh
                        h0 = o0 // Dh
                        nc.vector.tensor_copy(
                            out=V[st][:, h0 : h0 + nh, 0:Dh],
                            in_=pv[:, :w].rearrange("p (h d) -> p h d", d=Dh),
                        )
                    nc.vector.memset(V[st][:, :, Dh : Dh + 1], 1.0)

            # ---------------- attention, per head
            with (
                tc.tile_pool(name="pst", bufs=2, space=bass.MemorySpace.PSUM) as pst,
                tc.tile_pool(name="psg", bufs=1, space=bass.MemorySpace.PSUM) as psg,
                tc.tile_pool(name="psm", bufs=2, space=bass.MemorySpace.PSUM) as psm,
                tc.tile_pool(name="pctx", bufs=2, space=bass.MemorySpace.PSUM) as pctx,
            ):
                for h in range(H):
                    j, hp = h // 2, (h % 2) * Dh
                    qh = QT[j][hp : hp + Dh, :]
                    kh = KT[j][hp : hp + Dh, :]

                    # term scores^T [128 term keys, q] then exp -> et
                    et = ep.tile([128, AF], BF16, name="et", tag="et")
                    for s0, s1 in TERM_QCHUNKS:
                        w = s1 - s0
                        stp = pst.tile([128, 512], F32, name="stp", tag="st")
                        nc.tensor.matmul(
                            stp[:, :w],
                            lhsT=kh[:, AF:S],
                            rhs=qh[:, s0:s1],
                            start=True,
                            stop=True,
                        )
                        nc.scalar.activation(
                            out=et[:, s0:s1], in_=stp[:, :w], func=AF_.Exp, scale=SCALE
                        )

                    # sig scores^T: candidate c -> rows (c%2)*64, cols (c//2)*64
                    sga = psg.tile([128, 512], F32, name="sga", tag="sga")
                    sgb = psm.tile([128, 128], F32, name="sgb", tag="small")
                    for c in range(CDD):
                        row = (c % 2) * Dh
                        if c < 16:
                            dst = sga[row : row + Dh, (c // 2) * 64 : (c // 2) * 64 + 64]
                        else:
                            cb = (c // 2 - 8) * 64
                            dst = sgb[row : row + Dh, cb : cb + 64]
                        nc.tensor.matmul(
                            dst,
                            lhsT=kh[:, c * L : (c + 1) * L],
                            rhs=qh[:, c * L : (c + 1) * L],
                            start=True,
                            stop=True,
                        )
                    eg = ep.tile([128, 640], BF16, name="eg", tag="eg")
                    nc.scalar.activation(
                        out=eg[:, 0:512], in_=sga, func=AF_.Exp, scale=SCALE
                    )
                    nc.scalar.activation(
                        out=eg[:, 512:640], in_=sgb, func=AF_.Exp, scale=SCALE
                    )

                    # pst scores (symmetric: Q both sides) then exp -> epp
                    spp = psm.tile([128, 128], F32, name="spp", tag="small")
                    nc.tensor.matmul(
                        spp, lhsT=qh[:, AF:S], rhs=qh[:, AF:S], start=True, stop=True
                    )
                    epp = ep.tile([128, 128], BF16, name="epp", tag="ep")
                    nc.scalar.activation(out=epp, in_=spp, func=AF_.Exp, scale=SCALE)

                    # ctx: q-tile t -> [128, Dh+1]; col Dh = denominator
                    for tgrp in range(3):
                        n_in_grp = 4 if tgrp < 2 else 3
                        cps = pctx.tile([128, 4, Dh + 1], F32, name="cps", tag="ctx")
                        for ti in range(n_in_grp):
                            t = tgrp * 4 + ti
                            c0 = cps[:, ti, :]
                            if t < 10:
                                nc.tensor.matmul(
                                    c0,
                                    lhsT=et[:, t * 128 : (t + 1) * 128],
                                    rhs=V[NST - 1][:, h, :],
                                    start=True,
                                    stop=False,
                                )
                                nc.tensor.matmul(
                                    cps[0:64, ti, :],
                                    lhsT=eg[0:64, t * 64 : t * 64 + 64],
                                    rhs=V[t][0:64, h, :],
                                    start=False,
                                    stop=True,
                                )
                                nc.tensor.matmul(
                                    cps[64:128, ti, :],
                                    lhsT=eg[64:128, t * 64 : t * 64 + 64],
                                    rhs=V[t][64:128, h, :],
                                    start=False,
                                    stop=True,
                                )
                            else:
                                nc.tensor.matmul(
                                    c0,
                                    lhsT=epp,
                                    rhs=V[NST - 1][:, h, :],
                                    start=True,
                                    stop=True,
                                )
                            rc = mp.tile([128, 1], F32, name="rc", tag="rc")
                            nc.vector.reciprocal(out=rc, in_=cps[:, ti, Dh : Dh + 1])
                            nc.vector.tensor_scalar_mul(
                                out=OUT[t][:, h * Dh : (h + 1) * Dh],
                                in0=cps[:, ti, 0:Dh],
                                scalar1=rc,
                            )

            # ---------------- +bv, store
            for st in range(NST):
                nc.vector.tensor_add(out=OUT[st], in0=OUT[st], in1=bvb)
                nc.sync.dma_start(
                    out=out_d[st * 128 : (st + 1) * 128, :], in_=OUT[st]
                )

    _legalize_waits(nc)
    return nc


_NC = None


def _get_nc():
    global _NC
    if _NC is None:
        _NC = _build_program()
    return _NC


# -------------------------------------------------------------- host wrapper
def _prep_inputs(hidden_states, Wq, bq, Wk, Wv, bv):
    bf = ml_dtypes.bfloat16
    hs = np.asarray(hidden_states, dtype=np.float32)
    wq = np.asarray(Wq, dtype=np.float32)
    wk = np.asarray(Wk, dtype=np.float32)
    wv = np.asarray(Wv, dtype=np.float32)
    bq = np.asarray(bq, dtype=np.float32)
    bv = np.asarray(bv, dtype=np.float32)

    # W is [out, in]; device wants W^T = [in, out] (contraction on partitions)
    wqT = np.ascontiguousarray(wq.T).astype(bf)
    wkT = np.ascontiguousarray(wk.T).astype(bf)
    wvT = np.ascontiguousarray(wv.T).astype(bf)
    bq6 = np.ascontiguousarray(bq.reshape(NDC, 128, 1))
    bvb = np.broadcast_to(bv[None, :], (128, D)).copy()

    in_maps = []
    for b in range(B):
        xT = np.ascontiguousarray(hs[b].T).astype(bf)
        in_maps.append(
            {
                "xT": xT,
                "wqT": wqT,
                "wkT": wkT,
                "wvT": wvT,
                "bq": bq6,
                "bvb": bvb,
            }
        )
    return in_maps


def _enable_tracing():
    """This image lacks ``antenv.axon_hooks``; recreate the NTFF profile hook
    from the boot package's ctypes impl, and defang the artifact upload."""
    import types

    import antenv

    if "antenv.axon_hooks" not in sys.modules:
        from trn_agent_boot.trn_boot import _ntff_profile_via_ctypes

        hook = _ntff_profile_via_ctypes("/opt/axon/libaxon_pjrt.so")
        mod = types.ModuleType("antenv.axon_hooks")
        mod.get_axon_ntff_profile_hook = lambda: hook
        mod.set_axon_ntff_profile_hook = lambda h: None
        sys.modules["antenv.axon_hooks"] = mod
        antenv.axon_hooks = mod
    import concourse.bass_utils as bu

    bu.upload_artifacts = lambda tmpdir: tmpdir


def run(inputs, trace=False, tmpdir=None):
    """Returns (output [B,S,D] f32, BassKernelResults)."""
    if trace:
        _enable_tracing()
    assert int(inputs["num_heads"]) == H
    assert int(inputs["signal_length"]) == L
    assert int(inputs["cdd_size"]) == CDD
    assert int(inputs["term_num"]) == T
    nc = _get_nc()
    in_maps = _prep_inputs(
        inputs["hidden_states"],
        inputs["Wq"],
        inputs["bq"],
        inputs["Wk"],
        inputs["Wv"],
        inputs["bv"],
    )
    res = run_bass_kernel_spmd(
        nc, in_maps, list(range(B)), trace=trace, tmpdir=tmpdir
    )
    out = np.stack([res.results[c]["out"] for c in range(B)]).astype(np.float32)
    return out, res


def kernel(**inputs) -> np.ndarray:
    out, _ = run(inputs, trace=False)
    return out


# revision 6
# speedup vs baseline: 1.2901x; 1.2901x over previous
"""Sparse BERT self-attention (DeBERTa-style one-pass mask) on 8 Trainium2
NeuronCores. Data-parallel over batch: core b handles batch element b.

Shapes (hardcoded per problem spec):
  B=8, S=1408, D=768, H=12, Dh=64, L=64 (signal), CDD=20, T=128 (terms),
  AF = CDD*L = 1280.

Mask structure (training-mode one-pass, attention_mask==1 everywhere):
  - cdd query rows [0,1280): candidate c attends to its own 64 signal keys
    plus the 128 term keys  -> 192 keys per query.
  - term query rows [1280,1408): attend among the 128 term rows, with the
    *query* projection used for both sides (reference quirk).

Math notes (exact reassociations used by the kernel):
  - bk never enters: (Q+bq)·bk is constant over keys -> cancels in softmax.
  - bq IS added to Q (per-partition add in the Q^T layout).
  - bv is added after normalization (sum_k p = 1 -> +bv once).
  - exp without max-subtraction: |scores| <= ~5, safe in fp32 psum.
  - denominator: V tiles carry a ones-column per head; the ctx matmul
    accumulates sum(exp) into output column 64.
"""

import sys

sys.path.insert(0, "/opt/trn_rl_repo")

import numpy as np
import ml_dtypes

import concourse.bass as bass
import concourse.mybir as mybir
import concourse.tile as tile
from concourse.bass_utils import run_bass_kernel_spmd

# ---------------------------------------------------------------- constants
B, S, D = 8, 1408, 768
H, Dh = 12, 64
L, CDD, T = 64, 20, 128
AF = CDD * L  # 1280
NDC = D // 128  # 6 chunks of the contraction/output dim
NST = S // 128  # 11 s-tiles
SCALE = 1.0 / 8.0  # 1/sqrt(Dh)

BF16 = mybir.dt.bfloat16
F32 = mybir.dt.float32

QK_SCHUNKS = [(0, 512), (512, 1024), (1024, 1408)]  # s-chunks for Q/K proj
TERM_QCHUNKS = [(0, 512), (512, 1024), (1024, 1280)]  # cdd query chunks
V_OCHUNKS = [(0, 512), (512, 768)]  # output-dim chunks for V proj


# --------------------------------------------- walrus sem-wait legalization
def _legalize_waits(nc, max_waits=1):
    """This container's walrus rejects more than one sem wait per
    instruction. Hoist excess waits onto NOPs inserted just before the
    instruction on the same engine (engine streams execute in block order,
    so the conjunction of waits is preserved)."""
    from concourse import mybir

    k = 0
    for fn in nc.m.functions:
        for bb in fn.blocks:
            new_list = []
            changed = False
            for inst in bb.instructions:
                si = inst.sync_info
                waits = list(si.on_wait) if si is not None else []
                if len(waits) > max_waits:
                    changed = True
                    for w in waits[:-max_waits]:
                        nop = mybir.InstNoOp(name=f"waitsplit_{k}", ins=[], outs=[])
                        k += 1
                        nop.engine = inst.engine
                        nop.sync_info = mybir.SyncInfo(on_wait=[w], on_update=[])
                        new_list.append(nop)
                    inst.sync_info = mybir.SyncInfo(
                        on_wait=waits[-max_waits:], on_update=list(si.on_update)
                    )
                new_list.append(inst)
            if changed:
                bb.instructions = new_list


# ------------------------------------------------------------ bass program
def _build_program():
    nc = bass.Bass()
    AF_ = mybir.ActivationFunctionType
    ALU = mybir.AluOpType

    xT_d = nc.dram_tensor("xT", [D, S], BF16, kind="ExternalInput")
    wqT_d = nc.dram_tensor("wqT", [D, D], BF16, kind="ExternalInput")
    wkT_d = nc.dram_tensor("wkT", [D, D], BF16, kind="ExternalInput")
    wvT_d = nc.dram_tensor("wvT", [D, D], BF16, kind="ExternalInput")
    bq_d = nc.dram_tensor("bq", [NDC, 128, 1], F32, kind="ExternalInput")
    bvb_d = nc.dram_tensor("bvb", [128, D], F32, kind="ExternalInput")
    out_d = nc.dram_tensor("out", [S, D], F32, kind="ExternalOutput")

    with tile.TileContext(nc) as tc:
        with (
            tc.tile_pool(name="persist", bufs=1) as pp,
            tc.tile_pool(name="exps", bufs=2) as ep,
            tc.tile_pool(name="misc", bufs=4) as mp,
        ):
            # ---------------- input DMA
            xt = []
            for j in range(NDC):
                t = pp.tile([128, S], BF16, name=f"xt{j}", tag=f"xt{j}")
                nc.sync.dma_start(out=t, in_=xT_d[j * 128 : (j + 1) * 128, :])
                xt.append(t)
            wt = {}
            for nm, dram in (("q", wqT_d), ("k", wkT_d), ("v", wvT_d)):
                tiles = []
                for j in range(NDC):
                    t = pp.tile([128, D], BF16, name=f"w{nm}{j}", tag=f"w{nm}{j}")
                    nc.sync.dma_start(out=t, in_=dram[j * 128 : (j + 1) * 128, :])
                    tiles.append(t)
                wt[nm] = tiles
            bqt = []
            for j in range(NDC):
                t = pp.tile([128, 1], F32, name=f"bq{j}", tag=f"bq{j}")
                nc.sync.dma_start(out=t, in_=bq_d[j])
                bqt.append(t)
            bvb = pp.tile([128, D], F32, name="bvb", tag="bvb")
            nc.sync.dma_start(out=bvb, in_=bvb_d[:, :])

            QT = [pp.tile([128, S], BF16, name=f"qT{j}", tag=f"qT{j}") for j in range(NDC)]
            KT = [pp.tile([128, S], BF16, name=f"kT{j}", tag=f"kT{j}") for j in range(NDC)]
            # V tiles: [128, H, Dh+1]; column Dh holds ones (denominator).
            V = [pp.tile([128, H, Dh + 1], BF16, name=f"v{st}", tag=f"v{st}") for st in range(NST)]
            OUT = [pp.tile([128, D], F32, name=f"o{st}", tag=f"o{st}") for st in range(NST)]

            # ---------------- projections
            # Single PSUM budget (8 banks): proj 2, st 2, sga 1, small 1, ctx 2.
            pj = None
            with (
                tc.tile_pool(name="pproj", bufs=2, space=bass.MemorySpace.PSUM) as pj,
                tc.tile_pool(name="pst", bufs=2, space=bass.MemorySpace.PSUM) as pst,
                tc.tile_pool(name="psg", bufs=1, space=bass.MemorySpace.PSUM) as psg,
                tc.tile_pool(name="psm", bufs=1, space=bass.MemorySpace.PSUM) as psm,
                tc.tile_pool(name="pctx", bufs=2, space=bass.MemorySpace.PSUM) as pctx,
            ):
                for oc in range(NDC):
                    for s0, s1 in QK_SCHUNKS:
                        w = s1 - s0
                        pq = pj.tile([128, 512], F32, name="pq", tag="proj")
                        for dc in range(NDC):
                            nc.tensor.matmul(
                                pq[:, :w],
                                lhsT=wt["q"][dc][:, oc * 128 : (oc + 1) * 128],
                                rhs=xt[dc][:, s0:s1],
                                start=(dc == 0),
                                stop=(dc == NDC - 1),
                            )
                        # Q^T = psum + bq (per-partition), cast to bf16
                        nc.vector.tensor_scalar_add(
                            out=QT[oc][:, s0:s1], in0=pq[:, :w], scalar1=bqt[oc]
                        )
                        pk = pj.tile([128, 512], F32, name="pk", tag="proj")
                        for dc in range(NDC):
                            nc.tensor.matmul(
                                pk[:, :w],
                                lhsT=wt["k"][dc][:, oc * 128 : (oc + 1) * 128],
                                rhs=xt[dc][:, s0:s1],
                                start=(dc == 0),
                                stop=(dc == NDC - 1),
                            )
                        nc.scalar.activation(
                            out=KT[oc][:, s0:s1], in_=pk[:, :w], func=AF_.Copy
                        )
                for st in range(NST):
                    for o0, o1 in V_OCHUNKS:
                        w = o1 - o0
                        pv = pj.tile([128, 512], F32, name="pv", tag="proj")
                        for dc in range(NDC):
                            nc.tensor.matmul(
                                pv[:, :w],
                                lhsT=xt[dc][:, st * 128 : (st + 1) * 128],
                                rhs=wt["v"][dc][:, o0:o1],
                                start=(dc == 0),
                                stop=(dc == NDC - 1),
                            )
                        nh = w // Dh
                        h0 = o0 // Dh
                        nc.vector.tensor_copy(
                            out=V[st][:, h0 : h0 + nh, 0:Dh],
                            in_=pv[:, :w].rearrange("p (h d) -> p h d", d=Dh),
                        )
                    nc.vector.memset(V[st][:, :, Dh : Dh + 1], 1.0)

                # ---------------- scores + exp, per head (persistent exp tiles)
                ET, EG, EP = [], [], []
                for h in range(H):
                    j, hp = h // 2, (h % 2) * Dh
                    qh = QT[j][hp : hp + Dh, :]
                    kh = KT[j][hp : hp + Dh, :]

                    et = pp.tile([128, AF], BF16, name=f"et{h}", tag=f"et{h}")
                    for s0, s1 in TERM_QCHUNKS:
                        w = s1 - s0
                        stp = pst.tile([128, 512], F32, name="stp", tag="st")
                        nc.tensor.matmul(
                            stp[:, :w],
                            lhsT=kh[:, AF:S],
                            rhs=qh[:, s0:s1],
                            start=True,
                            stop=True,
                        )
                        nc.scalar.activation(
                            out=et[:, s0:s1], in_=stp[:, :w], func=AF_.Exp, scale=SCALE
                        )

                    sga = psg.tile([128, 512], F32, name="sga", tag="sga")
                    sgb = psm.tile([128, 128], F32, name="sgb", tag="small")
                    for c in range(CDD):
                        row = (c % 2) * Dh
                        if c < 16:
                            dst = sga[row : row + Dh, (c // 2) * 64 : (c // 2) * 64 + 64]
                        else:
                            cb = (c // 2 - 8) * 64
                            dst = sgb[row : row + Dh, cb : cb + 64]
                        nc.tensor.matmul(
                            dst,
                            lhsT=kh[:, c * L : (c + 1) * L],
                            rhs=qh[:, c * L : (c + 1) * L],
                            start=True,
                            stop=True,
                        )
                    eg = pp.tile([128, 640], BF16, name=f"eg{h}", tag=f"eg{h}")
                    nc.scalar.activation(
                        out=eg[:, 0:512], in_=sga, func=AF_.Exp, scale=SCALE
                    )
                    nc.scalar.activation(
                        out=eg[:, 512:640], in_=sgb, func=AF_.Exp, scale=SCALE
                    )

                    spp = psm.tile([128, 128], F32, name="spp", tag="small")
                    nc.tensor.matmul(
                        spp, lhsT=qh[:, AF:S], rhs=qh[:, AF:S], start=True, stop=True
                    )
                    epp = pp.tile([128, 128], BF16, name=f"ep{h}", tag=f"ep{h}")
                    nc.scalar.activation(out=epp, in_=spp, func=AF_.Exp, scale=SCALE)
                    ET.append(et)
                    EG.append(eg)
                    EP.append(epp)

                # ---------------- ctx: q-tile outer, head-groups of 4
                for t in range(NST):
                    for hg in range(3):
                        cps = pctx.tile([128, 4, Dh + 1], F32, name="cps", tag="ctx")
                        for hi in range(4):
                            h = hg * 4 + hi
                            c0 = cps[:, hi, :]
                            if t < 10:
                                nc.tensor.matmul(
                                    c0,
                                    lhsT=ET[h][:, t * 128 : (t + 1) * 128],
                                    rhs=V[NST - 1][:, h, :],
                                    start=True,
                                    stop=False,
                                )
                                nc.tensor.matmul(
                                    cps[0:64, hi, :],
                                    lhsT=EG[h][0:64, t * 64 : t * 64 + 64],
                                    rhs=V[t][0:64, h, :],
                                    start=False,
                                    stop=True,
                                )
                                nc.tensor.matmul(
                                    cps[64:128, hi, :],
                                    lhsT=EG[h][64:128, t * 64 : t * 64 + 64],
                                    rhs=V[t][64:128, h, :],
                                    start=False,
                                    stop=True,
                                )
                            else:
                                nc.tensor.matmul(
                                    c0,
                                    lhsT=EP[h],
                                    rhs=V[NST - 1][:, h, :],
                                    start=True,
                                    stop=True,
                                )
                        # batched normalize: one recip [128,4], one bcast mul
                        rc = mp.tile([128, 4], F32, name="rc", tag="rc")
                        nc.vector.reciprocal(out=rc, in_=cps[:, :, Dh : Dh + 1])
                        nc.vector.tensor_mul(
                            out=OUT[t][:, hg * 256 : (hg + 1) * 256].rearrange(
                                "p (h d) -> p h d", d=Dh
                            ),
                            in0=cps[:, :, 0:Dh],
                            in1=rc.to_broadcast([128, 4, Dh]),
                        )
                    nc.vector.tensor_add(out=OUT[t], in0=OUT[t], in1=bvb)
                    nc.sync.dma_start(
                        out=out_d[t * 128 : (t + 1) * 128, :], in_=OUT[t]
                    )

    _legalize_waits(nc)
    return nc


_NC = None


def _get_nc():
    global _NC
    if _NC is None:
        _NC = _build_program()
    return _NC


# -------------------------------------------------------------- host wrapper
def _prep_inputs(hidden_states, Wq, bq, Wk, Wv, bv):
    bf = ml_dtypes.bfloat16
    hs = np.asarray(hidden_states, dtype=np.float32)
    wq = np.asarray(Wq, dtype=np.float32)
    wk = np.asarray(Wk, dtype=np.float32)
    wv = np.asarray(Wv, dtype=np.float32)
    bq = np.asarray(bq, dtype=np.float32)
    bv = np.asarray(bv, dtype=np.float32)

    # W is [out, in]; device wants W^T = [in, out] (contraction on partitions)
    wqT = np.ascontiguousarray(wq.T).astype(bf)
    wkT = np.ascontiguousarray(wk.T).astype(bf)
    wvT = np.ascontiguousarray(wv.T).astype(bf)
    bq6 = np.ascontiguousarray(bq.reshape(NDC, 128, 1))
    bvb = np.broadcast_to(bv[None, :], (128, D)).copy()

    in_maps = []
    for b in range(B):
        xT = np.ascontiguousarray(hs[b].T).astype(bf)
        in_maps.append(
            {
                "xT": xT,
                "wqT": wqT,
                "wkT": wkT,
                "wvT": wvT,
                "bq": bq6,
                "bvb": bvb,
            }
        )
    return in_maps


def _enable_tracing():
    """This image lacks ``antenv.axon_hooks``; recreate the NTFF profile hook
    from the boot package's ctypes impl, and defang the artifact upload."""
    import types

    import antenv

    if "antenv.axon_hooks" not in sys.modules:
        from trn_agent_boot.trn_boot import _ntff_profile_via_ctypes

        hook = _ntff_profile_via_ctypes("/opt/axon/libaxon_pjrt.so")
        mod = types.ModuleType("antenv.axon_hooks")
        mod.get_axon_ntff_profile_hook = lambda: hook
        mod.set_axon_ntff_profile_hook = lambda h: None
        sys.modules["antenv.axon_hooks"] = mod
        antenv.axon_hooks = mod
    import concourse.bass_utils as bu

    bu.upload_artifacts = lambda tmpdir: tmpdir


def run(inputs, trace=False, tmpdir=None):
    """Returns (output [B,S,D] f32, BassKernelResults)."""
    if trace:
        _enable_tracing()
    assert int(inputs["num_heads"]) == H
    assert int(inputs["signal_length"]) == L
    assert int(inputs["cdd_size"]) == CDD
    assert int(inputs["term_num"]) == T
    nc = _get_nc()
    in_maps = _prep_inputs(
        inputs["hidden_states"],
        inputs["Wq"],
        inputs["bq"],
        inputs["Wk"],
        inputs["Wv"],
        inputs["bv"],
    )
    res = run_bass_kernel_spmd(
        nc, in_maps, list(range(B)), trace=trace, tmpdir=tmpdir
    )
    out = np.stack([res.results[c]["out"] for c in range(B)]).astype(np.float32)
    return out, res


def kernel(**inputs) -> np.ndarray:
    out, _ = run(inputs, trace=False)
    return out
